# revision 11
# baseline (speedup 1.0000x reference)
"""MaxPoolingAggregator kernel for 8x TRN2 NeuronCores.

Strategy (pure data parallel over nodes, 16384 nodes/core):
- Host pre-pass: cast neigh to fp8-e4m3 and self to bf16 (neigh path is
  ~0.5% of output magnitude so fp8 is invisible at the 2e-2 gate; self
  must stay bf16 -- fp8 self measures 2.8e-2) and lay both out
  pre-transposed in DRAM so every device DMA is a plain per-partition
  contiguous multi-KB load: neigh as x^T [128 d, (blk, j, n)] and self
  as self^T [128 d, 16384 n]. Output is stored [128 p, (blk, c)] and
  un-permuted on host, again so the store is one contiguous 2KB chunk
  per partition.
- Per 128-node block: 25 matmuls with the 128-col x^T j-slice as the
  FWL-accelerated fp8 stationary operand and bf16 W_mlp moving (~30ns
  each), so h lands NATURAL [128 nodes, 32] in PSUM; max-pool over the
  25 neighbor planes is a single 128-partition grouped tensor_reduce
  straight out of PSUM (bias+leaky commute with max). The reduce is the
  irreducible DVE item (~950ns/block); everything else is moved off DVE:
  - pool is reduced to bf16 and PE-transposed at 1 cyc/row (vs 2 for
    f32) into a shared PSUM bank batching 4 blocks;
  - bias-add runs once per 4 blocks on ACT ([32,512] batched);
  - leaky on the hidden is one batched DVE stt per 4 blocks;
  - stage-2 PSUM is ACT-copied into a 16-block staging tile and the
    final leaky is one batched DVE stt per 16 blocks feeding one
    contiguous store. (GpSimd was tried for these and is a dead end:
    walrus codegen rejects both TensorScalarPtr and TensorTensor on
    the Pool engine with NCC_IXCG966.)
- Software pipeline: stage-2 matmuls for a 4-block group are emitted a
  full group late so their hp dependency (ACT+GpSimd latency) is long
  resolved and the PE FIFO head never stalls; xt loads are 12 deep x 2
  blocks for ~27us of DMA runway.
- Hardware quirks baked in: ACT's Lrelu has a fixed 0.01 negative
  slope (alpha operand ignored) so leaky is mult+max; DoubleRow fp8 is
  a LOSS at moving-free-dim 32 (disables FWL) so plain fp8xbf16 is
  used; several ISA structs hold fewer sync-wait slots than Tile emits
  (_fix_transpose_waits hoists the excess onto carrier instructions).
"""

import sys

sys.path.insert(0, "/opt/trn_rl_repo")

import numpy as np
import ml_dtypes

BF16 = ml_dtypes.bfloat16
FP8 = ml_dtypes.float8_e4m3

N_CORES = 8
N_TOTAL = 131072
NEIGH = 25
DIN = 128
DH = 32
DO = 32
SHARD = N_TOTAL // N_CORES      # 16384 nodes per core
BLK = 128                       # nodes per block
NBLK = SHARD // BLK             # 128 blocks
SLOTS = BLK * NEIGH             # 3200 = (25 j, 128 n) slots per block
XBATCH = 2                      # blocks per xt load (6400B/partition)
XBUFS = 12                      # xt loads in flight (~27us runway)
ABATCH = 4                      # blocks per stage-a batch (fills a bank)
STORE_BATCH = 8                 # blocks per output store
SFT_CHUNKS = 4
SFT_CH = SHARD // SFT_CHUNKS
ALPHA = 0.02

_CACHE = {}


def _build():
    import concourse.bass as bass
    import concourse.mybir as mybir
    from concourse.tile import TileContext

    nc = bass.Bass()
    # x^T, host pre-arranged: row d holds [NBLK, 25 j, 128 n] contiguous
    neigh = nc.dram_tensor("neigh", [DIN, NBLK * SLOTS], mybir.dt.float8e4, kind="ExternalInput")
    # self^T, host pre-arranged: [128 d, 16384 n]
    selft = nc.dram_tensor("selft", [DIN, SHARD], mybir.dt.bfloat16, kind="ExternalInput")
    w_mlp = nc.dram_tensor("w_mlp", [DIN, DH], mybir.dt.float32, kind="ExternalInput")
    b_mlp = nc.dram_tensor("b_mlp", [DH], mybir.dt.float32, kind="ExternalInput")
    w_va = nc.dram_tensor("w_va", [DIN, DO], mybir.dt.float32, kind="ExternalInput")
    w_ng = nc.dram_tensor("w_ng", [DH, DO], mybir.dt.float32, kind="ExternalInput")
    identity = nc.dram_tensor("identity", [128, 128], mybir.dt.bfloat16, kind="ExternalInput")
    # out[p, b*32+c] = result[b*128+p, c]; host un-permutes
    out = nc.dram_tensor("out", [BLK, NBLK * DO], mybir.dt.float32, kind="ExternalOutput")

    ID = mybir.ActivationFunctionType.Identity
    COPY = mybir.ActivationFunctionType.Copy

    with TileContext(nc) as tc:
        with tc.tile_pool(name="const", bufs=1) as cpool, \
             tc.tile_pool(name="xt", bufs=XBUFS) as xtpool, \
             tc.tile_pool(name="sm", bufs=4) as smpool, \
             tc.tile_pool(name="hs", bufs=3) as hspool, \
             tc.tile_pool(name="st", bufs=4) as stpool, \
             tc.tile_pool(name="ob", bufs=2) as opool, \
             tc.tile_pool(name="ps", bufs=2, space="PSUM") as pspool, \
             tc.tile_pool(name="pst", bufs=2, space="PSUM") as pstpool, \
             tc.tile_pool(name="ps2", bufs=2, space="PSUM") as ps2pool:

            # ---- constants (scalar/ACT hwdge ring; sync ring is for xt) ----
            wm_f = cpool.tile([DIN, DH], mybir.dt.float32)
            nc.scalar.dma_start(wm_f[:], w_mlp[:])
            wm = cpool.tile([DIN, DH], mybir.dt.bfloat16)
            nc.vector.tensor_copy(wm[:], wm_f[:])
            wv_f = cpool.tile([DIN, DO], mybir.dt.float32)
            nc.scalar.dma_start(wv_f[:], w_va[:])
            wv = cpool.tile([DIN, DO], mybir.dt.bfloat16)
            nc.vector.tensor_copy(wv[:], wv_f[:])
            wn_f = cpool.tile([DH, DO], mybir.dt.float32)
            nc.scalar.dma_start(wn_f[:], w_ng[:])
            wn = cpool.tile([DH, DO], mybir.dt.bfloat16)
            nc.vector.tensor_copy(wn[:], wn_f[:])
            bm = cpool.tile([DH, 1], mybir.dt.float32)
            nc.scalar.dma_start(bm[:], b_mlp[:].rearrange("(h b) -> h b", b=1))
            ident = cpool.tile([128, 128], mybir.dt.bfloat16)
            nc.scalar.dma_start(ident[:], identity[:])
            # whole-shard self^T resident in SBUF (32 KB/partition),
            # loaded in 4 chunks so early stage-2 isn't blocked on the tail
            sfts = []
            for ci in range(SFT_CHUNKS):
                t = cpool.tile([DIN, SFT_CH], mybir.dt.bfloat16)
                nc.scalar.dma_start(t[:], selft[:, ci * SFT_CH:(ci + 1) * SFT_CH])
                sfts.append(t)

            # pipeline state
            pst = None           # current stage-a PSUM transpose batch
            pend_hp = None       # (hp, group) whose stage-2 is deferred
            stag = None          # stage-2 staging tile [128, 16*32] f32
            out_tile = None

            def stage2(hp, g):
                """Stage 2 for the 4 blocks of group g (hp long since ready)."""
                nonlocal stag, out_tile
                for u in range(ABATCH):
                    b = g * ABATCH + u
                    k = b % STORE_BATCH
                    if k == 0:
                        stag = stpool.tile([BLK, STORE_BATCH * DO],
                                           mybir.dt.float32, tag="stag")
                        out_tile = opool.tile([BLK, STORE_BATCH * DO],
                                              mybir.dt.float32, tag="ob")
                    ps2 = ps2pool.tile([BLK, DO], mybir.dt.float32, tag="st2")
                    c0 = (b * BLK) // SFT_CH
                    off = b * BLK - c0 * SFT_CH
                    nc.tensor.matmul(ps2[:], sfts[c0][:, off:off + BLK], wv[:],
                                     start=True, stop=False)
                    nc.tensor.matmul(ps2[:], hp[:, u * BLK:(u + 1) * BLK], wn[:],
                                     start=False, stop=True)
                    # PSUM -> staging on ACT; final leaky is one batched
                    # DVE stt per 16 blocks
                    nc.scalar.activation(stag[:, k * DO:(k + 1) * DO],
                                         ps2[:], COPY)
                    if k == STORE_BATCH - 1:
                        nc.vector.scalar_tensor_tensor(
                            out_tile[:], stag[:], ALPHA, stag[:],
                            op0=mybir.AluOpType.mult, op1=mybir.AluOpType.max)
                        b0 = b - (STORE_BATCH - 1)
                        # gpsimd/SWDGE ring: its sequencer is otherwise
                        # idle, so a store waiting on out_tile never
                        # stalls ACT compute or the xt load stream
                        nc.gpsimd.dma_start(
                            out[:, b0 * DO:(b + 1) * DO], out_tile[:])

            for bb in range(NBLK // XBATCH):
                # plain contiguous load of XBATCH blocks of x^T
                xt = xtpool.tile([128, XBATCH * SLOTS], mybir.dt.float8e4,
                                 tag="xt")
                nc.sync.dma_start(
                    xt[:], neigh[:, bb * XBATCH * SLOTS:(bb + 1) * XBATCH * SLOTS])

                for kk in range(XBATCH):
                    b = bb * XBATCH + kk
                    xb = xt[:, kk * SLOTS:(kk + 1) * SLOTS]

                    # stage 1: h_j = x_j @ W_mlp per neighbor plane j,
                    # landing natural [128 nodes, 32] at psum cols j*32
                    ps = pspool.tile([BLK, NEIGH * DH], mybir.dt.float32,
                                     tag="mlp")
                    for q in range(NEIGH):
                        nc.tensor.matmul(ps[:, q * DH:(q + 1) * DH],
                                         xb[:, q * BLK:(q + 1) * BLK], wm[:],
                                         start=True, stop=True)

                    # max-pool over the 25 neighbor planes: one
                    # 128-partition grouped reduce straight out of PSUM,
                    # down to bf16 so the PE transpose runs 1 cyc/row.
                    pool_sb = smpool.tile([BLK, DH], mybir.dt.bfloat16,
                                          tag="pool")
                    nc.vector.tensor_reduce(
                        pool_sb[:],
                        ps[:].rearrange("n (q h) -> n h q", q=NEIGH),
                        axis=mybir.AxisListType.X, op=mybir.AluOpType.max)

                    # pool^T via PE into the 4-block batch bank
                    a = b % ABATCH
                    if a == 0:
                        pst = pstpool.tile([DH, ABATCH * BLK],
                                           mybir.dt.bfloat16, tag="pt")
                    nc.tensor.transpose(pst[:, a * BLK:(a + 1) * BLK],
                                        pool_sb[:], ident[:])

                    if a == ABATCH - 1:
                        g = b // ABATCH
                        # batched bias on ACT, batched leaky on GpSimd
                        hpb = hspool.tile([DH, ABATCH * BLK],
                                          mybir.dt.float32, tag="hpb")
                        nc.scalar.activation(hpb[:], pst[:], ID, bias=bm[:])
                        hp = hspool.tile([DH, ABATCH * BLK],
                                         mybir.dt.bfloat16, tag="hp")
                        nc.vector.scalar_tensor_tensor(
                            hp[:], hpb[:], ALPHA, hpb[:],
                            op0=mybir.AluOpType.mult, op1=mybir.AluOpType.max)
                        # stage 2 one full group late so the PE FIFO head
                        # never waits on this group's hp
                        if pend_hp is not None:
                            stage2(*pend_hp)
                        pend_hp = (hp, g)

            stage2(*pend_hp)
    _fix_transpose_waits(nc)
    return nc


def _fix_transpose_waits(nc):
    """Several ISA structs (DMA_DIRECT2D_XPOSE, LDWEIGHTS/MATMULT) have
    fewer sync-wait slots than Tile sometimes emits. Hoist all waits
    beyond the first into standalone event-semaphore carrier
    instructions on the same engine queue (they execute in order ahead
    of the instruction, so semantics are preserved)."""
    import concourse.mybir as mybir

    uid = [0]
    for f in nc.m.functions:
        for bb in f.blocks:
            insts = list(bb.instructions)
            new_insts = []
            for inst in insts:
                si = inst.sync_info
                if si is not None and len(si.on_wait) > 1:
                    excess = list(si.on_wait[1:])
                    si.on_wait = [si.on_wait[0]]
                    for w in excess:
                        uid[0] += 1
                        carrier = mybir.InstEventSemaphore(
                            name=f"waitfix-{uid[0]}",
                            engine=inst.engine,
                            sync_info=mybir.SyncInfo(on_wait=[w], on_update=[]),
                        )
                        new_insts.append(carrier)
                new_insts.append(inst)
            bb.instructions = new_insts


def _get_nc():
    if "nc" not in _CACHE:
        _CACHE["nc"] = _build()
    return _CACHE["nc"]


def _prep_core(neigh_c, self_c):
    # x^T: [16384, 25, 128] -> [128 d, (NBLK b, 25 j, 128 n)]
    xt = neigh_c.astype(FP8).reshape(NBLK, BLK, NEIGH, DIN).transpose(3, 0, 2, 1)
    neigh_t = np.ascontiguousarray(xt).reshape(DIN, NBLK * SLOTS)
    self_t = np.ascontiguousarray(self_c.astype(BF16).T)
    return neigh_t, self_t


def run(inputs, trace=False, **kwargs):
    from concourse.bass_utils import run_bass_kernel_spmd

    nc = _get_nc()
    ident = np.eye(128, dtype=np.float32).astype(BF16)
    in_maps = []
    for c in range(N_CORES):
        sl = slice(c * SHARD, (c + 1) * SHARD)
        neigh_t, self_t = _prep_core(inputs["neigh_vecs"][sl],
                                     inputs["self_vecs"][sl])
        in_maps.append({
            "neigh": neigh_t,
            "selft": self_t,
            "w_mlp": inputs["W_mlp"],
            "b_mlp": inputs["b_mlp"],
            "w_va": inputs["W_va"],
            "w_ng": inputs["W_neigh"],
            "identity": ident,
        })
    res = run_bass_kernel_spmd(nc, in_maps, core_ids=list(range(N_CORES)),
                               trace=trace, **kwargs)
    outs = []
    for c in range(N_CORES):
        o = res.results[c]["out"]  # [128, NBLK*32]
        outs.append(o.reshape(BLK, NBLK, DO).transpose(1, 0, 2)
                     .reshape(SHARD, DO))
    full = np.concatenate(outs, axis=0)
    return full, res


def kernel(**inputs) -> np.ndarray:
    full, _ = run(inputs, trace=False)
    return full
